# revision 2
# baseline (speedup 1.0000x reference)
"""Trainium2 Bass kernel for nn_CNN_3496103379215 (normalized conv + argmax pool).

v2: float32r single-pass conv with merged-parity T (3.4x the v1 fp16 hi/lo
kernel in the timeline cost model: 787us -> 234us per core).

Pipeline per core (2 images, data-parallel across 8 cores):
  - 5x5 conv (1->2ch, pad 2) of conf ("denom") and data*conf ("nomin") as
    Toeplitz-band matmuls: contraction over 128 input rows (y-halo included),
    accumulating over 5 kernel columns (x-shift via free-dim AP offset).
  - float32r (1s8e11m, fp32 bits rounded to 11-bit mantissa by the PE)
    streams 1 column/cycle when the free dim is >=256 (vs 4 cycles for
    fp32), so den and nom each need ONE pass per x-tap vs 3/2 fp16 hi/lo
    passes in v1 -> 20 matmuls per (tile, ch) instead of 100.  fp32r dst
    APs must be even-aligned/even-width, so inputs are host-padded with 2
    zero x-border columns and every pass is full-width [0:512].
  - T [128, 126] merged-parity: columns 0..61 produce even output rows ->
    PSUM partitions 0..61, columns 64..125 produce odd rows -> partitions
    64..125.  One matmul covers both parities (full PE column utilization);
    each x-chunk (512 cols) gets its own PSUM bank (8 banks = 2 ch x 2
    kinds x 2 chunks, double-buffered across ch).
  - argmax of cout == argmax of denom (positive per-channel scale): the 2x2
    argmax-pool tournament (strict is_gt + copy_predicated, first-wins ties
    matching flat argmax order) runs on raw conv outputs at pooled
    resolution.  Even-x PSUM extraction runs on ScalarE (ACT) while VectorE
    does compare/select; the y-stage aligns odd-row lanes 64..125 to
    even-row lanes 0..61 with one sbuf->sbuf partition-shift DMA per
    (tile, ch) on the ACT HWDGE ring (input DMAs stay on the SP ring).
  - the c1 output scale 0.25/(wsum+eps) is folded into T (den and nom scale
    equally so x1 = nom/den and the argmax order are unchanged), making c1
    a direct DMA of the pooled den pack; x1 = nom/den + bias runs only at
    pooled resolution (DVE reciprocal + gpsimd mult + ACT bias-add).
  - software-pipelined: tile t+1 input DMAs + data*conf issue ahead of tile
    t pooling; engine assignment tuned against the TimelineSim cost model.

Measured on 8 axon-tunneled trn2 cores: rel err 1.06e-2 (x1, argmax
tie-flips from the 2^-12 fp32r rounding; gate 2e-2), 4.6e-5 (c1).
"""

import os
import numpy as np
from contextlib import ExitStack

EPS = 1e-20
H = W = 1024
HP = WP = 512
PER_CORE = 2          # images per core
N_CORES = 8
TILE_ROWS = 124       # output rows per y-tile (input rows = 128 incl +-2 halo)
N_TILES = 9           # ceil(1024 / 124)

_CACHE = {}


def _host_tensors(weight, bias):
    """Merged-parity T [128, 2*128] fp32 (per channel, 126 used cols + pad)
    and per-channel scalars [128, 4] (cols 0-1: bias, cols 2-3:
    0.25/(wsum+eps))."""
    weight = np.asarray(weight, np.float32)
    bias = np.asarray(bias, np.float32)
    wsum = weight.sum(axis=(1, 2, 3))
    ms = np.arange(62)
    tm_all = np.zeros((128, 10 * 128), np.float32)
    # fold the c1 output scale 0.25/(wsum+eps) into T: den and nom scale
    # equally, so x1 = nom/den is unchanged, argmax order is unchanged, and
    # c1 = maxpool(den_scaled) needs no epilogue op at all.
    sc = (0.25 / (wsum + EPS)).astype(np.float32)
    for ch in range(2):
        for dx in range(5):
            T = np.zeros((128, 128), np.float32)
            for parity in range(2):
                for dy in range(5):
                    T[2 * ms + parity + dy, 64 * parity + ms] = \
                        weight[ch, 0, dy, dx] * sc[ch]
            tm_all[:, (ch * 5 + dx) * 128:(ch * 5 + dx + 1) * 128] = T
    scb = np.zeros((128, 4), np.float32)
    scb[:, 0:2] = bias[None, :]
    scb[:, 2:4] = sc[None, :]
    return tm_all, scb


def _build_program(repeat=1):
    import concourse.bass as bass
    import concourse.tile as tile
    from concourse import bacc, mybir

    f32 = mybir.dt.float32
    f32r = mybir.dt.float32r
    u8 = mybir.dt.uint8
    nc = bacc.Bacc("TRN2", target_bir_lowering=False)

    data_ext = nc.declare_dram_parameter("data", [PER_CORE, 1, H, W + 4], f32r, isOutput=False)
    conf_ext = nc.declare_dram_parameter("conf", [PER_CORE, 1, H, W + 4], f32r, isOutput=False)
    tm_ext = nc.declare_dram_parameter("tm", [128, 10 * 128], f32r, isOutput=False)
    scb_ext = nc.declare_dram_parameter("scb", [128, 4], f32, isOutput=False)
    x1_ext = nc.declare_dram_parameter("x1", [PER_CORE, 2, HP, WP], f32, isOutput=True)
    c1_ext = nc.declare_dram_parameter("c1", [PER_CORE, 2, HP, WP], f32, isOutput=True)

    gt = mybir.AluOpType.is_gt
    mx = mybir.AluOpType.max

    with tile.TileContext(nc) as tc, ExitStack() as ctx:
        consts = ctx.enter_context(tc.tile_pool(name="consts", bufs=1))
        inp = ctx.enter_context(tc.tile_pool(name="inp", bufs=4))
        psum = ctx.enter_context(tc.tile_pool(name="psum", bufs=2, space="PSUM"))
        sx = ctx.enter_context(tc.tile_pool(name="sx", bufs=3))

        tm_t = consts.tile([128, 10 * 128], f32r)
        nc.sync.dma_start(out=tm_t[:, :], in_=tm_ext[:, :])
        scb_t = consts.tile([128, 4], f32)
        nc.sync.dma_start(out=scb_t[:, :], in_=scb_ext[:, :])

        _E_DC = nc.gpsimd
        _E_X1 = nc.gpsimd

        def tsl(ch, dx):
            i = ch * 5 + dx
            return tm_t[:, i * 128:i * 128 + 126]

        def prep(img, t):
            """Input DMAs + data*conf for one tile (issued one tile ahead)."""
            ys = TILE_ROWS * t
            r0 = ys - 2                      # first input row of tile (may be <0)
            cr0, cr1 = max(r0, 0), min(r0 + 128, H)

            # rhs tiles have 2 zero border columns each side so every
            # dx tap is a full-width matmul (fp32r dst needs even
            # offset/width; out-of-image taps read zeros)
            conf_t = inp.tile([128, W + 4], f32r, tag="conf")
            data_t = inp.tile([128, W + 4], f32r, tag="data")
            # inputs are host-padded with the 2 zero border cols, so the DMA
            # covers the full tile width; only the y-halo rows at image
            # top/bottom need zeroing (ACT memzero, off the DVE/DMA critical
            # path; 32-aligned partition window, DMA overwrites valid rows)
            for tt in (conf_t, data_t):
                if cr0 - r0 > 0:
                    nc.vector.memset(tt[0:32, :].bitcast(f32), 0.0)
                if cr1 - r0 < 128:
                    p0 = (cr1 - r0) // 32 * 32
                    nc.vector.memset(tt[p0:p0 + 32, :].bitcast(f32), 0.0)
            nc.sync.dma_start(out=conf_t[cr0 - r0:cr1 - r0, :],
                              in_=conf_ext[img, 0, cr0:cr1, :])
            nc.sync.dma_start(out=data_t[cr0 - r0:cr1 - r0, :],
                              in_=data_ext[img, 0, cr0:cr1, :])

            dc_t = inp.tile([128, W + 4], f32r, tag="dc")
            _E_DC.tensor_mul(dc_t[:, :], data_t[:, :], conf_t[:, :])
            return conf_t, dc_t

        def compute(img, t, conf_t, dc_t):
                n_valid = min(TILE_ROWS, H - TILE_ROWS * t)
                npool = (n_valid + 1) // 2
                pr0 = TILE_ROWS * t // 2

                # x1t is shared across channels so one DMA writes both
                x1t = sx.tile([128, 1024], f32, tag="x1t")
                x14 = x1t.rearrange("p (c k x) -> p c k x", c=2, x=256)
                pks = []

                for ch in range(2):
                    denA = psum.tile([128, 512], f32, tag="denA")
                    denB = psum.tile([128, 512], f32, tag="denB")
                    nomA = psum.tile([128, 512], f32, tag="nomA")
                    nomB = psum.tile([128, 512], f32, tag="nomB")
                    # all passes full width: out col x taps rhs tile col
                    # x + dx (border cols are zero).  All chunk-A matmuls
                    # issue before chunk-B so A-bank pooling overlaps B's
                    # matmuls.
                    for xoff, bkden, bknom in ((0, denA, nomA),
                                               (512, denB, nomB)):
                        for k, dx in enumerate(range(5)):
                            TT = tsl(ch, dx)
                            st, sp = k == 0, k == 4
                            for kind, bank in (("den", bkden), ("nom", bknom)):
                                rhs_t = (conf_t if kind == "den" else dc_t)
                                nc.tensor.matmul(
                                    bank[0:126, 0:512], TT,
                                    rhs_t[:, xoff + dx:xoff + dx + 512],
                                    start=st, stop=sp)

                    # ---- stage X: pool x-pairs (even/odd free columns) on all
                    # 128 partitions at once (even rows in lanes 0..61, odd in
                    # 64..125)
                    dAv = denA.rearrange("p (x two) -> p x two", two=2)
                    dBv = denB.rearrange("p (x two) -> p x two", two=2)
                    nAv = nomA.rearrange("p (x two) -> p x two", two=2)
                    nBv = nomB.rearrange("p (x two) -> p x two", two=2)

                    # per-channel pooled pack: col = chunk*512 + {0:cx|256:nx}
                    pk = sx.tile([128, 1024], f32, tag="pk")
                    mxm = sx.tile([128, 512], u8, tag="mxm")
                    cxA, nxA = pk[:, 0:256], pk[:, 256:512]
                    cxB, nxB = pk[:, 512:768], pk[:, 768:1024]
                    mA, mB = mxm[:, 0:256], mxm[:, 256:512]

                    # even-x slices PSUM->SBUF on ACT (ScalarE reads PSUM;
                    # DVE handles the compare/select ops in parallel).  A
                    # first: its banks stop while B's matmuls still stream.
                    nc.scalar.copy(cxA, dAv[:, :, 0])
                    nc.scalar.copy(nxA, nAv[:, :, 0])
                    nc.vector.tensor_tensor(mA, dAv[:, :, 1], cxA, op=gt)
                    nc.vector.tensor_tensor(cxA, cxA, dAv[:, :, 1], op=mx)
                    nc.vector.copy_predicated(nxA, mA, nAv[:, :, 1])
                    nc.scalar.copy(cxB, dBv[:, :, 0])
                    nc.scalar.copy(nxB, nBv[:, :, 0])
                    nc.vector.tensor_tensor(mB, dBv[:, :, 1], cxB, op=gt)
                    nc.vector.tensor_tensor(cxB, cxB, dBv[:, :, 1], op=mx)
                    nc.vector.copy_predicated(nxB, mB, nBv[:, :, 1])

                    # ---- partition shift: odd-row lanes 64..125 -> 0..61 ----
                    yo = sx.tile([128, 1024], f32, tag="yo")
                    nc.scalar.dma_start(out=yo[0:62, :], in_=pk[64:126, :])

                    # 3D views [p, chunk, sel, 256]: sel=0 -> cx, 1 -> nx
                    pk3 = pk.rearrange("p (k s x) -> p k s x", k=2, x=256)
                    yo3 = yo.rearrange("p (k s x) -> p k s x", k=2, x=256)
                    my3 = mxm.rearrange("p (k x) -> p k x", x=256)

                    # ---- stage Y: odd row beats even row only if strictly
                    # greater (first-wins ties match torch argmax) ----
                    nc.vector.tensor_tensor(my3[0:62], yo3[0:62, :, 0],
                                            pk3[0:62, :, 0], op=gt)
                    nc.vector.tensor_tensor(pk3[0:62, :, 0], pk3[0:62, :, 0],
                                            yo3[0:62, :, 0], op=mx)
                    nc.vector.copy_predicated(pk3[0:62, :, 1], my3[0:62],
                                              yo3[0:62, :, 1])

                    # ---- epilogue: x1 = nb / cb + bias ; c1 = cb (scale
                    # folded into T) ----
                    rv = sx.tile([128, 512], f32, tag="rv")
                    rv3 = rv.rearrange("p (k x) -> p k x", x=256)
                    nc.vector.reciprocal(rv3[0:62], pk3[0:62, :, 0])
                    _E_X1.tensor_tensor(x14[0:62, ch], pk3[0:62, :, 1],
                                        rv3[0:62],
                                        op=mybir.AluOpType.mult)
                    nc.scalar.add(x1t[0:62, ch * 512:ch * 512 + 512],
                                  x1t[0:62, ch * 512:ch * 512 + 512],
                                  scb_t[0:62, ch:ch + 1])
                    nc.scalar.dma_start(
                        out=c1_ext[img, ch, pr0:pr0 + npool, :],
                        in_=pk3[0:npool, :, 0])
                    pks.append(pk)

                # ---- x1 output DMA: one per tile (both channels); src
                # enumerates (row, ch, col), dram AP transposed to match ----
                for ch in range(2):
                    nc.scalar.dma_start(
                        out=x1_ext[img, ch, pr0:pr0 + npool, :],
                        in_=x1t[0:npool, ch * 512:ch * 512 + 512])

        # software pipeline: tile t+1's DMAs + data*conf are issued before
        # tile t's pool/epilogue so engine FIFOs never head-of-line block the
        # next tile's matmul operands
        units = [(img, t) for _ in range(repeat)
                 for img in range(PER_CORE) for t in range(N_TILES)]
        pending = prep(*units[0])
        for i, (img, t) in enumerate(units):
            cur = pending
            if i + 1 < len(units):
                pending = prep(*units[i + 1])
            compute(img, t, *cur)
    nc.compile()
    return nc


def kernel(data, conf, weight, bias):
    from concourse.bass_utils import run_bass_kernel_spmd

    data = np.asarray(data, np.float32)
    conf = np.asarray(conf, np.float32)
    # pad 2 zero columns on each x side (the kernel taps them for edge
    # columns; fp32r matmuls need full-width even-aligned windows)
    dpad = np.zeros(data.shape[:3] + (data.shape[3] + 4,), np.float32)
    cpad = np.zeros_like(dpad)
    dpad[..., 2:-2] = data
    cpad[..., 2:-2] = conf
    data, conf = dpad, cpad
    repeat = int(os.environ.get("BASS_KERNEL_REPEAT", "1"))
    key = ("nc", repeat)
    if key not in _CACHE:
        _CACHE[key] = _build_program(repeat)
    nc = _CACHE[key]

    tm, scb = _host_tensors(weight, bias)
    in_maps = []
    for c in range(N_CORES):
        sl = slice(c * PER_CORE, (c + 1) * PER_CORE)
        in_maps.append({"data": data[sl], "conf": conf[sl],
                        "tm": tm, "scb": scb})

    trace = bool(int(os.environ.get("BASS_KERNEL_TRACE", "0")))
    res = run_bass_kernel_spmd(nc, in_maps, list(range(N_CORES)), trace=trace)
    kernel.last_exec_time_ns = res.exec_time_ns

    x1 = np.concatenate([r["x1"] for r in res.results], axis=0)
    c1 = np.concatenate([r["c1"] for r in res.results], axis=0)
    return x1, c1


kernel.last_exec_time_ns = None
